# revision 3
# baseline (speedup 1.0000x reference)
"""GAT (2-layer graph attention) on 8 Trainium2 NeuronCores.

Node tables (256B rows: features | attention logits) are AllGather'd and
per-edge rows fetched with dma_gather. int16 gather indices cap tables at
32768 rows, so the 102400-row rank space splits into 4 windows; each
(dst-node, window) pair is a degree-sorted "virtual row" producing partial
softmax sums, combined by a second gather round. Softmax max-subtraction is
dropped (shift-invariant; logits are O(1)).
"""

import numpy as np
import ml_dtypes

bf16 = ml_dtypes.bfloat16

# ---------------- problem constants -----------------------------------
N = 100000
E = 1600000
NC = 8
F_IN = 512
H1, D1 = 8, 8
HD1 = H1 * D1
C = 40
NEG_SLOPE = 0.2
EPS = 1e-16

REAL = N // NC
BLOCKS = 100
SHARD = BLOCKS * 128
RANKS = NC * SHARD
WIN = 32768
NWIN = (RANKS + WIN - 1) // WIN
RW = 128                 # bf16 elems per table row (256B)
BATCH_KMAX = 64          # max sum-of-K per edge-gather call
CHUNK = 25               # combine blocks per chunk
ADST_GMAX = 64


def _set_dims(n, e, nc_, f_in, blocks, win, chunk, batch_kmax, adst_gmax):
    """Test hook: reconfigure sizes (must keep n % nc == 0 etc.)."""
    global N, E, NC, F_IN, REAL, BLOCKS, SHARD, RANKS, WIN, NWIN
    global CHUNK, BATCH_KMAX, ADST_GMAX
    N, E, NC, F_IN, BLOCKS, WIN = n, e, nc_, f_in, blocks, win
    CHUNK, BATCH_KMAX, ADST_GMAX = chunk, batch_kmax, adst_gmax
    REAL = N // NC
    SHARD = BLOCKS * 128
    RANKS = NC * SHARD
    NWIN = (RANKS + WIN - 1) // WIN
    assert REAL <= SHARD and WIN <= 32768


def _wrap_idx(flat):
    n = flat.shape[0]
    assert n % 16 == 0
    w16 = flat.reshape(n // 16, 16).T
    return np.tile(w16, (8, 1)).astype(np.int16)


def _pad_rel(w):
    wbase = w * WIN
    wend = min((w + 1) * WIN, RANKS)
    for c in range(NC):
        g0 = c * SHARD + REAL
        g1 = c * SHARD + SHARD - 1
        if g0 >= wbase and g1 < wend:
            return g0 - wbase
    raise AssertionError(f"no ghost row in window {w}")


def preprocess(edge_index):
    src = np.asarray(edge_index[0], np.int64)
    dst = np.asarray(edge_index[1], np.int64)
    loops = np.arange(N, dtype=np.int64)
    src = np.concatenate([src, loops])
    dst = np.concatenate([dst, loops])

    core = dst // REAL
    r_local = dst % REAL
    srcrank = (src // REAL) * SHARD + (src % REAL)
    w = srcrank // WIN
    rel = srcrank - w * WIN

    key = (core * NWIN + w) * REAL + r_local
    deg = np.bincount(key, minlength=NC * NWIN * REAL).reshape(NC, NWIN, REAL)

    vlists = {}
    nnz = np.zeros((NC, NWIN), int)
    for c in range(NC):
        for wi in range(NWIN):
            d = deg[c, wi]
            rs = np.nonzero(d)[0]
            order = np.argsort(-d[rs], kind="stable")
            rs = rs[order]
            vlists[(c, wi)] = (rs, d[rs])
            nnz[c, wi] = len(rs)

    G_w = [max(1, int(np.ceil(nnz[:, wi].max() / 128))) for wi in range(NWIN)]
    K_w = []
    for wi in range(NWIN):
        ks = np.zeros(G_w[wi], int)
        for c in range(NC):
            degs = vlists[(c, wi)][1]
            for g in range(G_w[wi]):
                if g * 128 < len(degs):
                    ks[g] = max(ks[g], degs[g * 128])
        ks = np.maximum(ks, 1)
        K_w.append(ks)

    batches_w = []
    for wi in range(NWIN):
        batches = []
        g0 = 0
        while g0 < G_w[wi]:
            g1, sk = g0, 0
            while g1 < G_w[wi] and sk + K_w[wi][g1] <= BATCH_KMAX:
                sk += K_w[wi][g1]
                g1 += 1
            assert g1 > g0
            batches.append((g0, g1, int(sk)))
            g0 = g1
        batches_w.append(batches)

    cumK_w = [np.concatenate([[0], np.cumsum(K_w[wi])]) for wi in range(NWIN)]
    pad_rel = [_pad_rel(wi) for wi in range(NWIN)]

    sortpos = np.full((NC, NWIN, REAL), -1, np.int64)
    for c in range(NC):
        for wi in range(NWIN):
            rs = vlists[(c, wi)][0]
            sortpos[c, wi, rs] = np.arange(len(rs))
    vpos = sortpos[core, w, r_local]
    order = np.argsort(key, kind="stable")
    k_in_row = np.empty(len(key), np.int64)
    sk_ = key[order]
    first = np.concatenate([[True], sk_[1:] != sk_[:-1]])
    starts = np.nonzero(first)[0]
    run_id = np.cumsum(first) - 1
    k_in_row[order] = np.arange(len(key)) - starts[run_id]

    gv = vpos // 128
    pv = vpos % 128

    idx_e_cores, idx_a_cores, idx_c_cores = [], [], []
    for c in range(NC):
        e_parts = []
        m_c = core == c
        for wi in range(NWIN):
            A = np.full((128, int(cumK_w[wi][-1])), pad_rel[wi], np.int64)
            m = m_c & (w == wi)
            col = cumK_w[wi][gv[m]] + k_in_row[m]
            A[pv[m], col] = rel[m]
            for (g0, g1, _sk) in batches_w[wi]:
                c0, c1 = int(cumK_w[wi][g0]), int(cumK_w[wi][g1])
                e_parts.append(_wrap_idx(A[:, c0:c1].T.reshape(-1)))
        idx_e_cores.append(np.concatenate(e_parts, axis=1))

        a_parts = []
        for wi in range(NWIN):
            rs = vlists[(c, wi)][0]
            R_flat = np.zeros(G_w[wi] * 128, np.int64)
            R_flat[: len(rs)] = rs
            R = R_flat.reshape(G_w[wi], 128).T
            g0 = 0
            while g0 < G_w[wi]:
                g1 = min(g0 + ADST_GMAX, G_w[wi])
                a_parts.append(_wrap_idx(R[:, g0:g1].T.reshape(-1)))
                g0 = g1
        idx_a_cores.append(np.concatenate(a_parts, axis=1))

        c_parts = []
        for wi in range(NWIN):
            zr = 128 * G_w[wi]
            Cidx = np.full(SHARD, zr, np.int64)
            rs = vlists[(c, wi)][0]
            vp = np.arange(len(rs))
            Cidx[rs] = (vp % 128) * G_w[wi] + (vp // 128)
            Cm = Cidx.reshape(BLOCKS, 128).T
            for b0 in range(0, BLOCKS, CHUNK):
                b1 = min(b0 + CHUNK, BLOCKS)
                c_parts.append(_wrap_idx(Cm[:, b0:b1].T.reshape(-1)))
        idx_c_cores.append(np.concatenate(c_parts, axis=1))

    struct = dict(
        G_w=G_w, K_w=K_w, batches_w=batches_w, cumK_w=cumK_w,
        idx_e_w=idx_e_cores[0].shape[1], idx_a_w=idx_a_cores[0].shape[1],
        idx_c_w=idx_c_cores[0].shape[1],
    )
    return struct, idx_e_cores, idx_a_cores, idx_c_cores


# -----------------------------------------------------------------------
def build(struct):
    import concourse.bacc as bacc
    import concourse.mybir as mybir
    import concourse.tile as tile
    from concourse.masks import make_identity

    F32 = mybir.dt.float32
    BF = mybir.dt.bfloat16
    I16 = mybir.dt.int16
    AX = mybir.AxisListType.X
    OP = mybir.AluOpType
    ACT = mybir.ActivationFunctionType

    G_w, K_w, batches_w = struct["G_w"], struct["K_w"], struct["batches_w"]
    cumK_w = struct["cumK_w"]
    KMAX = int(max(max(k) for k in K_w))
    FC = F_IN // 128

    nc = bacc.Bacc("TRN2", target_bir_lowering=False, debug=False,
                   num_devices=NC, num_swdge_queues=4)

    xt = nc.dram_tensor("xt", [F_IN, SHARD], F32, kind="ExternalInput").ap()
    w1 = nc.dram_tensor("w1", [F_IN, HD1], F32, kind="ExternalInput").ap()
    w2 = nc.dram_tensor("w2", [HD1, C], F32, kind="ExternalInput").ap()
    vec_in = {}
    for nm, width in [("atts1", HD1), ("attd1", HD1), ("b1", HD1),
                      ("atts2", C), ("attd2", C), ("b2", C)]:
        vec_in[nm] = nc.dram_tensor(nm, [1, width], F32,
                                    kind="ExternalInput").ap()
    idx_e = nc.dram_tensor("idx_e", [128, struct["idx_e_w"]], I16,
                           kind="ExternalInput").ap()
    idx_a = nc.dram_tensor("idx_a", [128, struct["idx_a_w"]], I16,
                           kind="ExternalInput").ap()
    idx_c = nc.dram_tensor("idx_c", [128, struct["idx_c_w"]], I16,
                           kind="ExternalInput").ap()
    out = nc.dram_tensor("out", [SHARD, C], F32, kind="ExternalOutput").ap()

    rg = [list(range(NC))]
    PT_rows = [128 * G_w[wi] + 1 for wi in range(NWIN)]
    PT_total = sum(PT_rows)
    PT_base = np.concatenate([[0], np.cumsum(PT_rows)]).astype(int)

    with tile.TileContext(nc) as tc:
        with (
            tc.tile_pool(name="dram", bufs=1, space="DRAM") as dpool,
            tc.tile_pool(name="setup", bufs=1) as sup,
            tc.tile_pool(name="psum0", bufs=2, space="PSUM") as psp,
        ):
            Rshard1 = dpool.tile([SHARD, RW], BF, tag="rs1")
            Rshard2 = dpool.tile([SHARD, RW], BF, tag="rs2")
            Rfull1 = dpool.tile([RANKS, RW], BF, tag="rf1")
            Rfull2 = dpool.tile([RANKS, RW], BF, tag="rf2")
            AdstT = dpool.tile([SHARD, RW], BF, tag="adt")
            Ptab1 = dpool.tile([PT_total, RW], BF, tag="pt1")
            Ptab2 = dpool.tile([PT_total, RW], BF, tag="pt2")

            ident = sup.tile([128, 128], F32)
            make_identity(nc, ident[:])
            ones_row = sup.tile([1, 128], F32)
            nc.vector.memset(ones_row[:], 1.0)

            w1_t = sup.tile([128, FC * HD1], F32)
            nc.sync.dma_start(
                w1_t[:].rearrange("p (c n) -> p c n", c=FC),
                w1.rearrange("(c p) n -> p c n", p=128),
            )
            w2_t = sup.tile([HD1, C], F32)
            nc.sync.dma_start(w2_t[:], w2[:, :])

            reps = {}
            for nm in ["atts1", "attd1", "b1", "atts2", "attd2", "b2"]:
                width = HD1 if nm in ("atts1", "attd1", "b1") else C
                v = sup.tile([1, width], F32, tag=f"v_{nm}")
                nc.sync.dma_start(v[:], vec_in[nm][:, :])
                ps = psp.tile([128, width], F32, tag="rep_ps")
                nc.tensor.matmul(out=ps[:], lhsT=ones_row[:], rhs=v[:],
                                 start=True, stop=True)
                r_ = sup.tile([128, width], F32, tag=f"rep_{nm}")
                nc.vector.tensor_copy(r_[:], ps[:])
                reps[nm] = r_

            ghost1 = sup.tile([128, 8], BF)
            nc.vector.memset(ghost1[:], -100.0)
            zrow = sup.tile([1, RW], BF)
            nc.vector.memset(zrow[:], 0.0)
            for wi in range(NWIN):
                zr = int(PT_base[wi]) + 128 * G_w[wi]
                nc.sync.dma_start(Ptab1[:][zr:zr + 1, :], zrow[:])
                nc.sync.dma_start(Ptab2[:][zr:zr + 1, :], zrow[:])

            # ---------------- dense layer 1 ----------------
            with (
                tc.tile_pool(name="d1", bufs=3) as dp,
                tc.tile_pool(name="d1p", bufs=2, space="PSUM") as dpp,
            ):
                for t in range(BLOCKS):
                    xtile = dp.tile([128, FC * 128], F32, tag="x")
                    nc.sync.dma_start(
                        xtile[:].rearrange("p (c n) -> p c n", c=FC),
                        xt.rearrange("(c p) n -> p c n", p=128)[
                            :, :, t * 128:(t + 1) * 128],
                    )
                    hps = dpp.tile([128, HD1], F32, tag="h")
                    for cc in range(FC):
                        nc.tensor.matmul(
                            out=hps[:],
                            lhsT=xtile[:].rearrange(
                                "p (c n) -> p c n", c=FC)[:, cc, :],
                            rhs=w1_t[:].rearrange(
                                "p (c n) -> p c n", c=FC)[:, cc, :],
                            start=(cc == 0), stop=(cc == FC - 1),
                        )
                    row = dp.tile([128, RW], BF, tag="row")
                    nc.vector.memset(row[:, 80:RW], 0.0)
                    nc.vector.tensor_copy(row[:, 0:HD1], hps[:])
                    asrc_f = dp.tile([128, H1], F32, tag="asrcf")
                    adst_f = dp.tile([128, H1], F32, tag="adstf")
                    tmp = dp.tile([128, HD1], F32, tag="tmp")
                    for nm, dst_ap in (("atts1", asrc_f), ("attd1", adst_f)):
                        nc.vector.tensor_tensor(
                            out=tmp[:], in0=hps[:], in1=reps[nm][:],
                            op=OP.mult)
                        nc.vector.tensor_reduce(
                            out=dst_ap[:],
                            in_=tmp[:].rearrange("p (h d) -> p h d", h=H1),
                            axis=AX, op=OP.add)
                    nc.vector.tensor_copy(row[:, 64:72], asrc_f[:])
                    nc.vector.tensor_copy(row[:, 72:80], adst_f[:])
                    nc.sync.dma_start(Rshard1[t * 128:(t + 1) * 128, :],
                                      row[:])
                    arow = dp.tile([128, RW], BF, tag="arow")
                    nc.vector.memset(arow[:, 8:RW], 0.0)
                    nc.vector.tensor_copy(arow[:, 0:8], adst_f[:])
                    nc.sync.dma_start(AdstT[t * 128:(t + 1) * 128, :],
                                      arow[:])

                for r0 in range(REAL, SHARD, 128):
                    r1 = min(r0 + 128, SHARD)
                    nc.sync.dma_start(Rshard1[r0:r1, 64:72],
                                      ghost1[: r1 - r0, :])

            nc.gpsimd.collective_compute(
                "AllGather", OP.bypass, replica_groups=rg,
                ins=[Rshard1.opt()], outs=[Rfull1.opt()])

            # ---------------- edge phase ----------------
            qn = [0]

            def edge_phase(Rfull, Ptab, layer):
                e_col = 0
                a_col = 0
                if layer == 1:
                    Hh, Dd, alo, dlo = H1, D1, 64, 0
                else:
                    Hh, Dd, alo, dlo = 1, C, 40, 8
                for wi in range(NWIN):
                    wbase = wi * WIN
                    wrows = min(WIN, RANKS - wbase)
                    Gn = G_w[wi]
                    with (
                        tc.tile_pool(name=f"ad{layer}{wi}", bufs=1) as apool,
                        tc.tile_pool(name=f"eg{layer}{wi}", bufs=2) as gp,
                        tc.tile_pool(name=f"ep{layer}{wi}", bufs=2) as ep,
                        tc.tile_pool(name=f"ix{layer}{wi}", bufs=2) as ixp,
                    ):
                        adstG = apool.tile([128, Gn * RW], BF, tag="adstG")
                        adstG_v = adstG[:].rearrange("p (g e) -> p g e", e=RW)
                        g0 = 0
                        while g0 < Gn:
                            g1 = min(g0 + ADST_GMAX, Gn)
                            nidx = (g1 - g0) * 128
                            ixa = ixp.tile([128, 8 * ADST_GMAX], I16,
                                           tag="ixa")
                            nc.sync.dma_start(
                                ixa[:, : nidx // 16],
                                idx_a[:, a_col: a_col + nidx // 16])
                            a_col += nidx // 16
                            nc.gpsimd.dma_gather(
                                out_ap=adstG_v[:, g0:g1, :],
                                in_ap=AdstT[:, :],
                                idxs_ap=ixa[:, : nidx // 16],
                                num_idxs=nidx, num_idxs_reg=nidx,
                                elem_size=RW, single_packet=False,
                                queue_num=qn[0] % 4)
                            qn[0] += 1
                            g0 = g1

                        for (g0, g1, sk) in batches_w[wi]:
                            nidx = 128 * sk
                            ixe = ixp.tile([128, 8 * BATCH_KMAX], I16,
                                           tag="ixe")
                            nc.sync.dma_start(
                                ixe[:, : nidx // 16],
                                idx_e[:, e_col: e_col + nidx // 16])
                            e_col += nidx // 16
                            G = gp.tile([128, BATCH_KMAX * RW], BF, tag="G")
                            Gv = G[:].rearrange("p (k e) -> p k e", e=RW)
                            nc.gpsimd.dma_gather(
                                out_ap=Gv[:, 0:sk, :],
                                in_ap=Rfull[:][wbase: wbase + wrows, :],
                                idxs_ap=ixe[:, : nidx // 16],
                                num_idxs=nidx, num_idxs_reg=nidx,
                                elem_size=RW, single_packet=False,
                                queue_num=qn[0] % 4)
                            qn[0] += 1

                            eT = ep.tile([128, BATCH_KMAX * Hh], F32,
                                         tag="eT")
                            eV = eT[:].rearrange("p (k h) -> p k h", h=Hh)
                            pT = ep.tile([128, BATCH_KMAX * Hh], BF, tag="pT")
                            pV = pT[:].rearrange("p (k h) -> p k h", h=Hh)
                            pb = ep.tile([128, BATCH_KMAX * RW], BF, tag="pb")
                            pbV = pb[:].rearrange("p (g e) -> p g e", e=RW)
                            ng = g1 - g0
                            if layer == 1:
                                nc.vector.memset(pbV[:, 0:ng, 80:RW], 0.0)
                            else:
                                nc.vector.memset(pbV[:, 0:ng, 40:64], 0.0)
                                nc.vector.memset(pbV[:, 0:ng, 66:RW], 0.0)

                            for gi in range(g0, g1):
                                K = int(K_w[wi][gi])
                                co = int(cumK_w[wi][gi] - cumK_w[wi][g0])
                                nc.vector.tensor_tensor(
                                    out=eV[:, co:co + K, :],
                                    in0=Gv[:, co:co + K, alo:alo + Hh],
                                    in1=adstG_v[:, gi:gi + 1, dlo:dlo + Hh]
                                        .to_broadcast([128, K, Hh]),
                                    op=OP.add)
                            ee = ep.tile([128, BATCH_KMAX * Hh], F32,
                                         tag="ee")
                            nc.vector.tensor_scalar_mul(
                                ee[:, : sk * Hh], eT[:, : sk * Hh], NEG_SLOPE)
                            nc.vector.tensor_tensor(
                                out=eT[:, : sk * Hh], in0=eT[:, : sk * Hh],
                                in1=ee[:, : sk * Hh], op=OP.max)
                            nc.scalar.activation(
                                pT[:, : sk * Hh], eT[:, : sk * Hh], ACT.Exp)

                            for gi in range(g0, g1):
                                K = int(K_w[wi][gi])
                                co = int(cumK_w[wi][gi] - cumK_w[wi][g0])
                                sW = ep.tile([128, Hh], F32, tag="sW")
                                nc.vector.tensor_reduce(
                                    out=sW[:],
                                    in_=pV[:, co:co + K, :]
                                        .transpose([0, 2, 1]),
                                    axis=AX, op=OP.add)
                                msg = ep.tile([128, KMAX * HD1], F32,
                                              tag="msg")
                                msgV = msg[:].rearrange(
                                    "p (k f) -> p k f", k=KMAX)
                                nc.vector.tensor_tensor(
                                    out=msgV[:, 0:K, 0:Hh * Dd].rearrange(
                                        "p k (h d) -> p k h d", h=Hh),
                                    in0=Gv[:, co:co + K, 0:Hh * Dd].rearrange(
                                        "p k (h d) -> p k h d", h=Hh),
                                    in1=pV[:, co:co + K, :].unsqueeze(3)
                                        .to_broadcast([128, K, Hh, Dd]),
                                    op=OP.mult)
                                uW = ep.tile([128, HD1], F32, tag="uW")
                                nc.vector.tensor_reduce(
                                    out=uW[:, 0:Hh * Dd],
                                    in_=msgV[:, 0:K, 0:Hh * Dd]
                                        .transpose([0, 2, 1]),
                                    axis=AX, op=OP.add)
                                nc.vector.tensor_copy(
                                    pbV[:, gi - g0, 0:Hh * Dd],
                                    uW[:, 0:Hh * Dd])
                                nc.vector.tensor_copy(
                                    pbV[:, gi - g0, 64:64 + 2 * Hh]
                                        .bitcast(F32), sW[:])
                            nc.sync.dma_start(
                                Ptab[:][int(PT_base[wi]):
                                        int(PT_base[wi]) + 128 * Gn, :]
                                .rearrange("(p g) e -> p g e", p=128)
                                [:, g0:g1, :],
                                pbV[:, 0: g1 - g0, :])

            edge_phase(Rfull1, Ptab1, 1)

            # ------------- combine helpers -------------
            def combine_chunks(Ptab, body, cp, cxp, tagp):
                for b0 in range(0, BLOCKS, CHUNK):
                    b1 = min(b0 + CHUNK, BLOCKS)
                    nb = b1 - b0
                    CWs = []
                    for wi in range(NWIN):
                        nidx = nb * 128
                        ixc = cxp.tile([128, 8 * CHUNK], I16,
                                       tag=f"ixc{tagp}{wi}")
                        off = (wi * BLOCKS + b0) * 128 // 16
                        nc.sync.dma_start(
                            ixc[:, : nidx // 16],
                            idx_c[:, off: off + nidx // 16])
                        CW = cp.tile([128, CHUNK * RW], BF,
                                     tag=f"cw{tagp}{wi}")
                        nc.gpsimd.dma_gather(
                            out_ap=CW[:].rearrange(
                                "p (b e) -> p b e", e=RW)[:, 0:nb, :],
                            in_ap=Ptab[:][int(PT_base[wi]):
                                          int(PT_base[wi]) + PT_rows[wi], :],
                            idxs_ap=ixc[:, : nidx // 16],
                            num_idxs=nidx, num_idxs_reg=nidx,
                            elem_size=RW, single_packet=False,
                            queue_num=wi)
                        CWs.append(CW[:].rearrange("p (b e) -> p b e", e=RW))
                    body(b0, b1, CWs)

            def add4(cp, CWs, nb, lo, hi, ftag):
                width = hi - lo
                acc = cp.tile([128, CHUNK * width], F32, tag=f"acc{ftag}")
                t0 = cp.tile([128, CHUNK * width], F32, tag=f"t0{ftag}")
                accV = acc[:].rearrange("p (b f) -> p b f", f=width)
                t0V = t0[:].rearrange("p (b f) -> p b f", f=width)
                nc.vector.tensor_tensor(
                    out=accV[:, 0:nb], in0=CWs[0][:, 0:nb, lo:hi],
                    in1=CWs[1][:, 0:nb, lo:hi], op=OP.add)
                if NWIN > 2:
                    nc.vector.tensor_tensor(
                        out=t0V[:, 0:nb], in0=CWs[2][:, 0:nb, lo:hi],
                        in1=CWs[3][:, 0:nb, lo:hi], op=OP.add)
                    nc.vector.tensor_tensor(
                        out=accV[:, 0:nb], in0=accV[:, 0:nb],
                        in1=t0V[:, 0:nb], op=OP.add)
                return accV

            def add4_f32(cp, CWs, nb, lo, nf, ftag):
                acc = cp.tile([128, CHUNK * nf], F32, tag=f"acs{ftag}")
                t0 = cp.tile([128, CHUNK * nf], F32, tag=f"ts{ftag}")
                accV = acc[:].rearrange("p (b f) -> p b f", f=nf)
                t0V = t0[:].rearrange("p (b f) -> p b f", f=nf)
                nc.vector.tensor_tensor(
                    out=accV[:, 0:nb],
                    in0=CWs[0][:, 0:nb, lo:lo + 2 * nf].bitcast(F32),
                    in1=CWs[1][:, 0:nb, lo:lo + 2 * nf].bitcast(F32),
                    op=OP.add)
                if NWIN > 2:
                    nc.vector.tensor_tensor(
                        out=t0V[:, 0:nb],
                        in0=CWs[2][:, 0:nb, lo:lo + 2 * nf].bitcast(F32),
                        in1=CWs[3][:, 0:nb, lo:lo + 2 * nf].bitcast(F32),
                        op=OP.add)
                    nc.vector.tensor_tensor(
                        out=accV[:, 0:nb], in0=accV[:, 0:nb],
                        in1=t0V[:, 0:nb], op=OP.add)
                return accV

            # ---------------- combine L1 + dense layer 2 ----------------
            with (
                tc.tile_pool(name="c1", bufs=2) as cp,
                tc.tile_pool(name="c1x", bufs=2) as cxp,
                tc.tile_pool(name="c1p", bufs=2, space="PSUM") as cpp,
            ):
                def c1_body(b0, b1, CWs):
                    nb = b1 - b0
                    UcV = add4(cp, CWs, nb, 0, HD1, "u1")
                    ScV = add4_f32(cp, CWs, nb, 64, H1, "s1")
                    for b in range(b0, b1):
                        bi = b - b0
                        rinv = cp.tile([128, H1], F32, tag="rinv")
                        nc.vector.tensor_scalar(
                            out=rinv[:], in0=ScV[:, bi], scalar1=EPS,
                            scalar2=None, op0=OP.add)
                        nc.vector.reciprocal(rinv[:], rinv[:])
                        o1 = cp.tile([128, HD1], F32, tag="o1")
                        nc.vector.tensor_tensor(
                            out=o1[:].rearrange("p (h d) -> p h d", h=H1),
                            in0=UcV[:, bi].rearrange("p (h d) -> p h d",
                                                     h=H1),
                            in1=rinv[:].unsqueeze(2).to_broadcast(
                                [128, H1, D1]),
                            op=OP.mult)
                        nc.vector.tensor_tensor(
                            out=o1[:], in0=o1[:], in1=reps["b1"][:],
                            op=OP.add)
                        mn = cp.tile([128, HD1], F32, tag="mn")
                        nc.vector.tensor_scalar(
                            out=mn[:], in0=o1[:], scalar1=0.0, scalar2=None,
                            op0=OP.min)
                        ex = cp.tile([128, HD1], F32, tag="ex")
                        nc.scalar.activation(ex[:], mn[:], ACT.Exp)
                        nc.vector.tensor_scalar(
                            out=o1[:], in0=o1[:], scalar1=0.0, scalar2=None,
                            op0=OP.max)
                        nc.vector.tensor_tensor(
                            out=o1[:], in0=o1[:], in1=ex[:], op=OP.add)
                        nc.vector.tensor_scalar(
                            out=o1[:], in0=o1[:], scalar1=-1.0, scalar2=None,
                            op0=OP.add)
                        tp = cpp.tile([128, 128], F32, tag="tp")
                        nc.tensor.transpose(
                            out=tp[0:HD1, :], in_=o1[:], identity=ident[:])
                        eT_ = cp.tile([HD1, 128], F32, tag="eT2")
                        nc.vector.tensor_copy(eT_[:], tp[0:HD1, :])
                        h2p = cpp.tile([128, C], F32, tag="h2p")
                        nc.tensor.matmul(out=h2p[:], lhsT=eT_[:],
                                         rhs=w2_t[:], start=True, stop=True)
                        row2 = cp.tile([128, RW], BF, tag="row2")
                        nc.vector.memset(row2[:, 42:RW], 0.0)
                        nc.vector.tensor_copy(row2[:, 0:C], h2p[:])
                        tmp2 = cp.tile([128, C], F32, tag="tmp2")
                        a2 = cp.tile([128, 1], F32, tag="a2")
                        for nm, lo in (("atts2", 40), ("attd2", 41)):
                            nc.vector.tensor_tensor(
                                out=tmp2[:], in0=h2p[:], in1=reps[nm][:],
                                op=OP.mult)
                            nc.vector.tensor_reduce(
                                out=a2[:], in_=tmp2[:], axis=AX, op=OP.add)
                            nc.vector.tensor_copy(row2[:, lo:lo + 1], a2[:])
                        nc.sync.dma_start(
                            Rshard2[b * 128:(b + 1) * 128, :], row2[:])
                        adr = cp.tile([128, 1], BF, tag="adr")
                        nc.vector.tensor_copy(adr[:], row2[:, 41:42])
                        nc.sync.dma_start(
                            AdstT[b * 128:(b + 1) * 128, 8:9], adr[:])

                combine_chunks(Ptab1, c1_body, cp, cxp, "a")

                gz = cp.tile([128, 42], BF, tag="gz")
                nc.vector.memset(gz[:, 0:40], 0.0)
                nc.vector.memset(gz[:, 40:41], -100.0)
                nc.vector.memset(gz[:, 41:42], 0.0)
                for r0 in range(REAL, SHARD, 128):
                    r1 = min(r0 + 128, SHARD)
                    nc.sync.dma_start(Rshard2[r0:r1, 0:42],
                                      gz[: r1 - r0, :])

            nc.gpsimd.collective_compute(
                "AllGather", OP.bypass, replica_groups=rg,
                ins=[Rshard2.opt()], outs=[Rfull2.opt()])

            edge_phase(Rfull2, Ptab2, 2)

            # ---------------- combine L2 + log_softmax ----------------
            with (
                tc.tile_pool(name="c2", bufs=2) as cp2,
                tc.tile_pool(name="c2x", bufs=2) as cxp2,
            ):
                def c2_body(b0, b1, CWs):
                    nb = b1 - b0
                    UcV = add4(cp2, CWs, nb, 0, C, "u2")
                    ScV = add4_f32(cp2, CWs, nb, 64, 1, "s2")
                    for b in range(b0, b1):
                        bi = b - b0
                        rinv = cp2.tile([128, 1], F32, tag="rinv2")
                        nc.vector.tensor_scalar(
                            out=rinv[:], in0=ScV[:, bi], scalar1=EPS,
                            scalar2=None, op0=OP.add)
                        nc.vector.reciprocal(rinv[:], rinv[:])
                        o2 = cp2.tile([128, C], F32, tag="o2")
                        nc.vector.tensor_tensor(
                            out=o2[:], in0=UcV[:, bi],
                            in1=rinv[:].to_broadcast([128, C]), op=OP.mult)
                        nc.vector.tensor_tensor(
                            out=o2[:], in0=o2[:], in1=reps["b2"][:],
                            op=OP.add)
                        mx = cp2.tile([128, 1], F32, tag="mx")
                        nc.vector.tensor_reduce(
                            out=mx[:], in_=o2[:], axis=AX, op=OP.max)
                        nc.vector.tensor_tensor(
                            out=o2[:], in0=o2[:],
                            in1=mx[:].to_broadcast([128, C]),
                            op=OP.subtract)
                        ex2 = cp2.tile([128, C], F32, tag="ex2")
                        nc.scalar.activation(ex2[:], o2[:], ACT.Exp)
                        ss = cp2.tile([128, 1], F32, tag="ss")
                        nc.vector.tensor_reduce(
                            out=ss[:], in_=ex2[:], axis=AX, op=OP.add)
                        nc.scalar.activation(ss[:], ss[:], ACT.Ln)
                        nc.vector.tensor_tensor(
                            out=o2[:], in0=o2[:],
                            in1=ss[:].to_broadcast([128, C]),
                            op=OP.subtract)
                        nc.sync.dma_start(out[b * 128:(b + 1) * 128, :],
                                          o2[:])

                combine_chunks(Ptab2, c2_body, cp2, cxp2, "b")

    nc.compile()
    return nc


_CACHE = {}


def _in_maps(inputs, idx_e, idx_a, idx_c):
    x = np.asarray(inputs["x"], np.float32)
    maps = []
    for c in range(NC):
        xs = np.zeros((F_IN, SHARD), np.float32)
        xs[:, :REAL] = x[c * REAL:(c + 1) * REAL].T
        maps.append({
            "xt": xs,
            "w1": np.asarray(inputs["W1"], np.float32),
            "w2": np.asarray(inputs["W2"], np.float32),
            "atts1": np.asarray(inputs["att_src1"], np.float32).reshape(1, HD1),
            "attd1": np.asarray(inputs["att_dst1"], np.float32).reshape(1, HD1),
            "b1": np.asarray(inputs["b1"], np.float32).reshape(1, HD1),
            "atts2": np.asarray(inputs["att_src2"], np.float32).reshape(1, C),
            "attd2": np.asarray(inputs["att_dst2"], np.float32).reshape(1, C),
            "b2": np.asarray(inputs["b2"], np.float32).reshape(1, C),
            "idx_e": idx_e[c], "idx_a": idx_a[c], "idx_c": idx_c[c],
        })
    return maps


def kernel(**inputs):
    from concourse import bass_utils

    struct, idx_e, idx_a, idx_c = preprocess(inputs["edge_index"])
    key = (struct["idx_e_w"], struct["idx_a_w"], struct["idx_c_w"],
           tuple(struct["G_w"]))
    if key not in _CACHE:
        _CACHE[key] = build(struct)
    nc = _CACHE[key]

    maps = _in_maps(inputs, idx_e, idx_a, idx_c)
    res = bass_utils.run_bass_kernel_spmd(nc, maps, core_ids=list(range(NC)))
    out = np.concatenate(
        [res.results[c]["out"][:REAL] for c in range(NC)], axis=0)
    return out.astype(np.float32)


if __name__ == "__main__":
    rng = np.random.default_rng(0)
    ei = np.stack([rng.integers(0, N, E), rng.integers(0, N, E)])
    struct, *_ = preprocess(ei.astype(np.int64))
    print("G_w:", struct["G_w"])
    print("K sums:", [int(k.sum()) for k in struct["K_w"]])
    print("widths:", struct["idx_e_w"], struct["idx_a_w"], struct["idx_c_w"])
